# revision 7
# baseline (speedup 1.0000x reference)
"""Multi-head self-attention (B=4, N=2048, D=1024, H=16) on 8 trn2 NeuronCores.

Sharding: 8 shards = (batch, query-half).  Core c handles batch c//2 and query
rows [(c%2)*1024, (c%2)*1024+1024).  Each core receives its batch's z with the
rows rolled so that its query rows come first; rolling permutes the key/value
sequence order, which attention output is invariant to.  K/V are computed for
the full 2048-row sequence on both cores of a batch pair.

Single fused phase, software-pipelined by head pair (hp) so that ACT (exp)
overlaps the next head pair's projections:
  - z^T via PE transposes into a resident bf16 ztc [128, 8, 2048].
  - Per hp: K^T/Q^T projections (bf16 weights, host-cast) into ping-pong SBUF
    slots; V' = [V|1] (all heads) woven into the early hp windows.
  - Per head (sequential): scores via 64-contraction matmuls, exp on ACT
    ([128,1024] activates), PV with the ones column producing the softmax
    denominator in row 64.  PV accumulates 8 key-chunks in PSUM, then is
    flushed/accumulated into an SBUF fp32 tile to keep PSUM pressure at
    2 banks (8 total: scores 2x2 + pv 2 + proj 2).
  - Normalization: reciprocal of the denominator row + gpsimd partition
    broadcast + DVE multiply into bf16 attnT.
  - Final projection attnT^T @ w_o + b_o at the tail through recycled PSUM.
"""

import os
import sys

_TRN_REPO = "/opt/trn_rl_repo"
if os.path.isdir(_TRN_REPO) and _TRN_REPO not in sys.path:
    sys.path.insert(0, _TRN_REPO)

import ml_dtypes
import numpy as np

import concourse.bass as bass  # noqa: E402
import concourse.mybir as mybir  # noqa: E402
from concourse import bacc  # noqa: E402
from concourse.bass_utils import run_bass_kernel_spmd  # noqa: E402
from concourse.masks import make_identity  # noqa: E402
from concourse.tile import TileContext  # noqa: E402

F32 = mybir.dt.float32
BF16 = mybir.dt.bfloat16
MULT = mybir.AluOpType.mult
ADD = mybir.AluOpType.add
EXP = mybir.ActivationFunctionType.Exp

N_CORES = 8
B, N, D = 4, 2048, 1024
H, HD = 16, 64
NQ = N // 2  # query rows per core
P = 128
DC = D // P  # 8 din/dout chunks of 128
NKC = N // P  # 16 key chunks of 128
HP = H // 2  # 8 head pairs
SCALE = 1.0 / 8.0  # 1/sqrt(HD)


def _build():
    nc = bacc.Bacc("TRN2", target_bir_lowering=False, debug=False,
                   num_devices=N_CORES)
    z_d = nc.declare_dram_parameter("z", [N, D], F32, isOutput=False)
    wq_d = nc.declare_dram_parameter("w_q", [D, D], BF16, isOutput=False)
    wk_d = nc.declare_dram_parameter("w_k", [D, D], BF16, isOutput=False)
    wv_d = nc.declare_dram_parameter("w_v", [D, D], BF16, isOutput=False)
    wo_d = nc.declare_dram_parameter("w_o", [D, D], BF16, isOutput=False)
    bo_d = nc.declare_dram_parameter("b_o", [D], F32, isOutput=False)
    out_d = nc.declare_dram_parameter("out", [NQ, D], F32, isOutput=True)

    with TileContext(nc) as tc:
        with tc.tile_pool(name="const", bufs=1) as constp, \
             tc.tile_pool(name="wp", bufs=1) as wp, \
             tc.tile_pool(name="zin", bufs=2) as zinp, \
             tc.tile_pool(name="zt", bufs=1) as ztp, \
             tc.tile_pool(name="kq", bufs=1) as kqp, \
             tc.tile_pool(name="vpool", bufs=1) as vpool, \
             tc.tile_pool(name="es", bufs=3) as esp, \
             tc.tile_pool(name="at", bufs=1) as atp, \
             tc.tile_pool(name="pva", bufs=2) as pvap, \
             tc.tile_pool(name="rr", bufs=1) as rrp, \
             tc.tile_pool(name="ot", bufs=2) as otp, \
             tc.tile_pool(name="psc", bufs=2, space="PSUM") as pscp, \
             tc.tile_pool(name="ppv", bufs=1, space="PSUM") as ppvp, \
             tc.tile_pool(name="pac", bufs=2, space="PSUM") as pacp:

            # ---- constants & resident tensors ----
            ident = constp.tile([P, P], F32)
            make_identity(nc, ident)

            wk_sb = wp.tile([P, DC, D], BF16)
            nc.scalar.dma_start(wk_sb[:], wk_d.rearrange("(c p) o -> p c o", p=P))
            wq_sb = wp.tile([P, DC, D], BF16)
            nc.scalar.dma_start(wq_sb[:], wq_d.rearrange("(c p) o -> p c o", p=P))
            wv_sb = wp.tile([P, DC, D], BF16)
            nc.scalar.dma_start(wv_sb[:], wv_d.rearrange("(c p) o -> p c o", p=P))
            wo_sb = wp.tile([P, DC, D], BF16)
            nc.scalar.dma_start(wo_sb[:], wo_d.rearrange("(c p) o -> p c o", p=P))
            bo_sb = wp.tile([1, D], F32)
            nc.scalar.dma_start(bo_sb[:], bo_d[None, :])
            bo_bc = wp.tile([P, D], F32)
            nc.gpsimd.partition_broadcast(bo_bc[:], bo_sb[:])

            ztc = ztp.tile([P, DC, N], BF16)            # z^T, din-major
            ktz = kqp.tile([P, 2, N], BF16)             # K^T hp ping-pong
            qtz = kqp.tile([P, 2, NQ], BF16)            # Q^T hp ping-pong
            vp = vpool.tile([P, NKC, H, HD + 1], BF16)  # V' = [V | 1]
            nc.vector.memset(vp[:, :, :, HD], 1.0)
            attnT = atp.tile([P, DC, NQ], BF16)         # normalized attn^T

            # ---- z load + PE transpose (8 chunks of 256 seq rows) ----
            for c8 in range(8):
                zin = zinp.tile([P, 2, D], F32, tag="zin")
                nc.sync.dma_start(
                    zin[:],
                    z_d[c8 * 256:(c8 + 1) * 256, :].rearrange(
                        "(r p) d -> p r d", p=P))
                for half in range(2):
                    ps = pscp.tile([P, NQ], F32, tag="sc")
                    for dcl in range(4):
                        dc = half * 4 + dcl
                        for r in range(2):
                            nc.tensor.transpose(
                                ps[:, dcl * 256 + r * P: dcl * 256 + (r + 1) * P],
                                zin[:, r, dc * P:(dc + 1) * P],
                                ident[:])
                    nc.vector.tensor_copy(
                        ztc[:, half * 4:(half + 1) * 4, c8 * 256:(c8 + 1) * 256],
                        ps.rearrange("p (c s) -> p c s", s=256))

            # ---- projection chunk helpers ----
            def k_chunk(hp, s):
                ps = pacp.tile([P, 512], F32, tag="acc")
                for dc in range(DC):
                    nc.tensor.matmul(
                        ps[:],
                        lhsT=wk_sb[:, dc, hp * P:(hp + 1) * P],
                        rhs=ztc[:, dc, s * 512:(s + 1) * 512],
                        start=(dc == 0), stop=(dc == DC - 1))
                nc.vector.tensor_copy(ktz[:, hp % 2, s * 512:(s + 1) * 512], ps[:])

            def q_chunk(hp, s):
                ps = pacp.tile([P, 512], F32, tag="acc")
                for dc in range(DC):
                    nc.tensor.matmul(
                        ps[:],
                        lhsT=wq_sb[:, dc, hp * P:(hp + 1) * P],
                        rhs=ztc[:, dc, s * 512:(s + 1) * 512],
                        start=(dc == 0), stop=(dc == DC - 1))
                nc.vector.tensor_copy(qtz[:, hp % 2, s * 512:(s + 1) * 512], ps[:])

            def v_chunk(kc4, oc2):
                ps = pacp.tile([P, 512], F32, tag="acc")
                for dc in range(DC):
                    nc.tensor.matmul(
                        ps[:],
                        lhsT=ztc[:, dc, kc4 * P:(kc4 + 1) * P],
                        rhs=wv_sb[:, dc, oc2 * 512:(oc2 + 1) * 512],
                        start=(dc == 0), stop=(dc == DC - 1))
                nc.vector.tensor_copy(
                    vp[:, kc4, oc2 * 8:(oc2 + 1) * 8, 0:HD],
                    ps.rearrange("p (h d) -> p h d", d=HD))

            # ---- lead-in: hp0's K^T/Q^T ----
            for s in range(4):
                k_chunk(0, s)
            for s in range(2):
                q_chunk(0, s)

            # V chunks for oc2=1 (heads 8-15), woven into hp1-3 windows
            v1_list = [(kc4, 1) for kc4 in range(NKC)]
            v1_split = {1: v1_list[0:6], 2: v1_list[6:11], 3: v1_list[11:16]}

            # ---- main loop over head pairs, heads sequential ----
            for hp in range(HP):
                slot = hp % 2

                fillers = []
                if hp + 1 < HP:
                    for s in range(4):
                        fillers.append(lambda s=s, n=hp + 1: k_chunk(n, s))
                    for s in range(2):
                        fillers.append(lambda s=s, n=hp + 1: q_chunk(n, s))
                for kc4, oc2 in v1_split.get(hp, []):
                    fillers.append(lambda a=kc4, b=oc2: v_chunk(a, b))

                for head in range(2):
                    h = 2 * hp + head
                    po = 64 * head
                    pv = ppvp.tile([HD + 1, NQ], F32, tag="pv")
                    pvacc = pvap.tile([HD + 1, NQ], F32, tag="pvacc")
                    for kc in range(NKC):
                        if hp == 0 and head == 0:
                            v_chunk(kc, 0)
                        ps = pscp.tile([P, NQ], F32, tag="sc")
                        for qc in range(2):
                            nc.tensor.matmul(
                                ps[:, qc * 512:(qc + 1) * 512],
                                lhsT=ktz[po:po + 64, slot, kc * P:(kc + 1) * P],
                                rhs=qtz[po:po + 64, slot, qc * 512:(qc + 1) * 512])
                        es = esp.tile([P, NQ], BF16, tag="es")
                        nc.scalar.activation(es[:], ps[:], EXP, scale=SCALE)
                        st = kc % 8 == 0
                        sp = kc % 8 == 7
                        for qc in range(2):
                            nc.tensor.matmul(
                                pv[:, qc * 512:(qc + 1) * 512],
                                lhsT=vp[:, kc, h, :],
                                rhs=es[:, qc * 512:(qc + 1) * 512],
                                start=st, stop=sp)
                        if kc == 7:
                            nc.vector.tensor_copy(pvacc[:], pv[:])
                        if kc % 2 == 1 and fillers:
                            fillers.pop(0)()
                    nc.vector.tensor_tensor(pvacc[:], pv[:], pvacc[:], ADD)

                    # normalization
                    rec = rrp.tile([1, NQ], F32, tag="rec")
                    nc.vector.reciprocal(rec[:], pvacc[HD:HD + 1, :])
                    rb = rrp.tile([64, NQ], F32, tag="rb")
                    nc.gpsimd.partition_broadcast(rb[:], rec[:])
                    nc.vector.tensor_tensor(
                        attnT[po:po + 64, hp, :],
                        pvacc[0:HD, :], rb[:], MULT)
                while fillers:
                    fillers.pop(0)()

            # ---- tail: out = attnT^T @ w_o + b_o ----
            for q8 in range(NQ // P):
                if q8 % 3 == 2:
                    ps = ppvp.tile([P, NQ], F32, tag="pv")
                else:
                    ps = pscp.tile([P, NQ], F32, tag="sc")
                for oc2 in range(2):
                    for dc in range(DC):
                        nc.tensor.matmul(
                            ps[:, oc2 * 512:(oc2 + 1) * 512],
                            lhsT=attnT[:, dc, q8 * P:(q8 + 1) * P],
                            rhs=wo_sb[:, dc, oc2 * 512:(oc2 + 1) * 512],
                            start=(dc == 0), stop=(dc == DC - 1))
                for oc2 in range(2):
                    ot = otp.tile([P, 512], F32, tag="ot")
                    nc.vector.tensor_tensor(
                        ot[:], ps[:, oc2 * 512:(oc2 + 1) * 512],
                        bo_bc[:, oc2 * 512:(oc2 + 1) * 512], ADD)
                    nc.sync.dma_start(
                        out_d[q8 * P:(q8 + 1) * P,
                              oc2 * 512:(oc2 + 1) * 512], ot[:])

    nc.compile()
    return nc


_NC_CACHE = None


def _get_nc():
    global _NC_CACHE
    if _NC_CACHE is None:
        _NC_CACHE = _build()
    return _NC_CACHE


def _run(z, w_q, w_k, w_v, w_o, b_o, **spmd_kwargs):
    z = np.ascontiguousarray(np.asarray(z, dtype=np.float32))
    w_q = np.ascontiguousarray(np.asarray(w_q, dtype=np.float32)).astype(
        ml_dtypes.bfloat16)
    w_k = np.ascontiguousarray(np.asarray(w_k, dtype=np.float32)).astype(
        ml_dtypes.bfloat16)
    w_v = np.ascontiguousarray(np.asarray(w_v, dtype=np.float32)).astype(
        ml_dtypes.bfloat16)
    w_o = np.ascontiguousarray(np.asarray(w_o, dtype=np.float32)).astype(
        ml_dtypes.bfloat16)
    b_o = np.ascontiguousarray(np.asarray(b_o, dtype=np.float32))
    assert z.shape == (B, N, D)

    if not spmd_kwargs.get("trace"):
        # A stray BASS_TRACE in the environment would route through the NTFF
        # hook (absent in this image) and crash; force the no-trace path.
        os.environ["BASS_NEVER_TRACE"] = "1"

    nc = _get_nc()
    in_maps = []
    for c in range(N_CORES):
        b = c // 2
        off = (c % 2) * NQ
        zc = np.ascontiguousarray(np.concatenate([z[b, off:], z[b, :off]], axis=0))
        in_maps.append({"z": zc, "w_q": w_q, "w_k": w_k, "w_v": w_v,
                        "w_o": w_o, "b_o": b_o})

    res = run_bass_kernel_spmd(nc, in_maps, core_ids=list(range(N_CORES)),
                               **spmd_kwargs)
    out = np.empty((B, N, D), dtype=np.float32)
    for c in range(N_CORES):
        b = c // 2
        off = (c % 2) * NQ
        out[b, off:off + NQ, :] = res.results[c]["out"]
    return out, res


def kernel(z, w_q, w_k, w_v, w_o, b_o):
    out, _ = _run(z, w_q, w_k, w_v, w_o, b_o)
    return out


# revision 10
# speedup vs baseline: 1.0656x; 1.0656x over previous
"""Multi-head self-attention (B=4, N=2048, D=1024, H=16) on 8 trn2 NeuronCores.

Sharding: 8 shards = (batch, query-half).  Core c handles batch c//2 and query
rows [(c%2)*1024, (c%2)*1024+1024).  Each core receives its batch's z with the
rows rolled so that its query rows come first; rolling permutes the key/value
sequence order, which attention output is invariant to.  K/V are computed for
the full 2048-row sequence on both cores of a batch pair.

Single fused phase, software-pipelined by head pair (hp) so that ACT (exp)
overlaps the next head pair's projections:
  - z^T via PE transposes into a resident bf16 ztc [128, 8, 2048].
  - Per hp: K^T/Q^T projections (bf16 weights, host-cast) into ping-pong SBUF
    slots; V' = [V|1] (all heads) woven into the early hp windows.
  - Per head (sequential): scores via 64-contraction matmuls, exp on ACT
    ([128,1024] activates), PV with the ones column producing the softmax
    denominator in row 64.  PV accumulates 8 key-chunks in PSUM, then is
    flushed/accumulated into an SBUF fp32 tile to keep PSUM pressure at
    2 banks (8 total: scores 2x2 + pv 2 + proj 2).
  - Normalization: reciprocal of the denominator row + gpsimd partition
    broadcast + DVE multiply into bf16 attnT.
  - Final projection attnT^T @ w_o + b_o at the tail through recycled PSUM.
"""

import os
import sys

_TRN_REPO = "/opt/trn_rl_repo"
if os.path.isdir(_TRN_REPO) and _TRN_REPO not in sys.path:
    sys.path.insert(0, _TRN_REPO)

import ml_dtypes
import numpy as np

import concourse.bass as bass  # noqa: E402
import concourse.mybir as mybir  # noqa: E402
from concourse import bacc  # noqa: E402
from concourse.bass_utils import run_bass_kernel_spmd  # noqa: E402
from concourse.masks import make_identity  # noqa: E402
from concourse.tile import TileContext  # noqa: E402

F32 = mybir.dt.float32
BF16 = mybir.dt.bfloat16
MULT = mybir.AluOpType.mult
ADD = mybir.AluOpType.add
EXP = mybir.ActivationFunctionType.Exp

N_CORES = 8
B, N, D = 4, 2048, 1024
H, HD = 16, 64
NQ = N // 2  # query rows per core
P = 128
DC = D // P  # 8 din/dout chunks of 128
NKC = N // P  # 16 key chunks of 128
HP = H // 2  # 8 head pairs
SCALE = 1.0 / 8.0  # 1/sqrt(HD)


def _build():
    nc = bacc.Bacc("TRN2", target_bir_lowering=False, debug=False,
                   num_devices=N_CORES)
    z_d = nc.declare_dram_parameter("z", [N, D], F32, isOutput=False)
    wq_d = nc.declare_dram_parameter("w_q", [D, D], BF16, isOutput=False)
    wk_d = nc.declare_dram_parameter("w_k", [D, D], BF16, isOutput=False)
    wv_d = nc.declare_dram_parameter("w_v", [D, D], BF16, isOutput=False)
    wo_d = nc.declare_dram_parameter("w_o", [D, D], BF16, isOutput=False)
    bo_d = nc.declare_dram_parameter("b_o", [D], F32, isOutput=False)
    out_d = nc.declare_dram_parameter("out", [NQ, D], F32, isOutput=True)

    with TileContext(nc) as tc:
        with tc.tile_pool(name="const", bufs=1) as constp, \
             tc.tile_pool(name="wp", bufs=1) as wp, \
             tc.tile_pool(name="zin", bufs=2) as zinp, \
             tc.tile_pool(name="zt", bufs=1) as ztp, \
             tc.tile_pool(name="kq", bufs=1) as kqp, \
             tc.tile_pool(name="vpool", bufs=1) as vpool, \
             tc.tile_pool(name="es", bufs=4) as esp, \
             tc.tile_pool(name="at", bufs=1) as atp, \
             tc.tile_pool(name="pva", bufs=2) as pvap, \
             tc.tile_pool(name="rr", bufs=1) as rrp, \
             tc.tile_pool(name="ot", bufs=1) as otp, \
             tc.tile_pool(name="psc", bufs=2, space="PSUM") as pscp, \
             tc.tile_pool(name="ppv", bufs=1, space="PSUM") as ppvp, \
             tc.tile_pool(name="pac", bufs=2, space="PSUM") as pacp:

            # ---- constants & resident tensors ----
            ident = constp.tile([P, P], F32)
            make_identity(nc, ident)

            wk_sb = wp.tile([P, DC, D], BF16)
            nc.scalar.dma_start(wk_sb[:], wk_d.rearrange("(c p) o -> p c o", p=P))
            wq_sb = wp.tile([P, DC, D], BF16)
            nc.scalar.dma_start(wq_sb[:], wq_d.rearrange("(c p) o -> p c o", p=P))
            wv_sb = wp.tile([P, DC, D], BF16)
            nc.scalar.dma_start(wv_sb[:], wv_d.rearrange("(c p) o -> p c o", p=P))
            wo_sb = wp.tile([P, DC, D], BF16)
            nc.scalar.dma_start(wo_sb[:], wo_d.rearrange("(c p) o -> p c o", p=P))
            bo_sb = wp.tile([1, D], F32)
            nc.scalar.dma_start(bo_sb[:], bo_d[None, :])
            bo_bc = wp.tile([P, D], F32)
            nc.gpsimd.partition_broadcast(bo_bc[:], bo_sb[:])

            ztc = ztp.tile([P, DC, N], BF16)            # z^T, din-major
            ktz = kqp.tile([P, 2, N], BF16)             # K^T hp ping-pong
            qtz = kqp.tile([P, 2, NQ], BF16)            # Q^T hp ping-pong
            vp = vpool.tile([P, NKC, H, HD + 1], BF16)  # V' = [V | 1]
            nc.vector.memset(vp[:, :, :, HD], 1.0)
            attnT = atp.tile([P, DC, NQ], BF16)         # normalized attn^T

            # ---- z load + PE transpose (8 chunks of 256 seq rows) ----
            for c8 in range(8):
                zin = zinp.tile([P, 2, D], F32, tag="zin")
                nc.sync.dma_start(
                    zin[:],
                    z_d[c8 * 256:(c8 + 1) * 256, :].rearrange(
                        "(r p) d -> p r d", p=P))
                for half in range(2):
                    ps = pscp.tile([P, NQ], F32, tag="sc")
                    for dcl in range(4):
                        dc = half * 4 + dcl
                        for r in range(2):
                            nc.tensor.transpose(
                                ps[:, dcl * 256 + r * P: dcl * 256 + (r + 1) * P],
                                zin[:, r, dc * P:(dc + 1) * P],
                                ident[:])
                    nc.vector.tensor_copy(
                        ztc[:, half * 4:(half + 1) * 4, c8 * 256:(c8 + 1) * 256],
                        ps.rearrange("p (c s) -> p c s", s=256))

            # ---- projection chunk helpers ----
            def k_chunk(hp, s):
                ps = pacp.tile([P, 512], F32, tag="acc")
                for dc in range(DC):
                    nc.tensor.matmul(
                        ps[:],
                        lhsT=wk_sb[:, dc, hp * P:(hp + 1) * P],
                        rhs=ztc[:, dc, s * 512:(s + 1) * 512],
                        start=(dc == 0), stop=(dc == DC - 1))
                nc.vector.tensor_copy(ktz[:, hp % 2, s * 512:(s + 1) * 512], ps[:])

            def q_chunk(hp, s):
                ps = pacp.tile([P, 512], F32, tag="acc")
                for dc in range(DC):
                    nc.tensor.matmul(
                        ps[:],
                        lhsT=wq_sb[:, dc, hp * P:(hp + 1) * P],
                        rhs=ztc[:, dc, s * 512:(s + 1) * 512],
                        start=(dc == 0), stop=(dc == DC - 1))
                nc.vector.tensor_copy(qtz[:, hp % 2, s * 512:(s + 1) * 512], ps[:])

            def v_chunk(kc4, oc2):
                ps = pacp.tile([P, 512], F32, tag="acc")
                for dc in range(DC):
                    nc.tensor.matmul(
                        ps[:],
                        lhsT=ztc[:, dc, kc4 * P:(kc4 + 1) * P],
                        rhs=wv_sb[:, dc, oc2 * 512:(oc2 + 1) * 512],
                        start=(dc == 0), stop=(dc == DC - 1))
                nc.vector.tensor_copy(
                    vp[:, kc4, oc2 * 8:(oc2 + 1) * 8, 0:HD],
                    ps.rearrange("p (h d) -> p h d", d=HD))

            # ---- lead-in: hp0's K^T/Q^T ----
            for s in range(4):
                k_chunk(0, s)
            for s in range(2):
                q_chunk(0, s)

            # V chunks for oc2=1 (heads 8-15), woven into hp1-3 windows
            v1_list = [(kc4, 1) for kc4 in range(NKC)]
            v1_split = {1: v1_list[0:6], 2: v1_list[6:11], 3: v1_list[11:16]}

            # ---- main loop over head pairs, heads sequential ----
            for hp in range(HP):
                slot = hp % 2

                fillers = []
                if hp + 1 < HP:
                    for s in range(4):
                        fillers.append(lambda s=s, n=hp + 1: k_chunk(n, s))
                    for s in range(2):
                        fillers.append(lambda s=s, n=hp + 1: q_chunk(n, s))
                for kc4, oc2 in v1_split.get(hp, []):
                    fillers.append(lambda a=kc4, b=oc2: v_chunk(a, b))

                for head in range(2):
                    h = 2 * hp + head
                    po = 64 * head
                    pv = ppvp.tile([HD + 1, NQ], F32, tag="pv")
                    pvacc = pvap.tile([HD + 1, NQ], F32, tag="pvacc")
                    es_hist = {}

                    def emit_pv(kc, h=h, pv=pv, pvacc=pvacc, es_hist=es_hist):
                        es = es_hist.pop(kc)
                        st = kc % 8 == 0
                        sp = kc % 8 == 7
                        for qc in range(2):
                            nc.tensor.matmul(
                                pv[:, qc * 512:(qc + 1) * 512],
                                lhsT=vp[:, kc, h, :],
                                rhs=es[:, qc * 512:(qc + 1) * 512],
                                start=st, stop=sp)
                        if kc == 7:
                            nc.vector.tensor_copy(pvacc[:], pv[:])

                    # software-pipelined: PV lags scores/exp by 2 key-chunks
                    # so the PE never head-of-line blocks on ACT.
                    for kc in range(NKC):
                        if hp == 0 and head == 0:
                            v_chunk(kc, 0)
                        if kc >= 2:
                            emit_pv(kc - 2)
                        if kc % 2 == 1 and fillers:
                            fillers.pop(0)()
                        ps = pscp.tile([P, NQ], F32, tag="sc")
                        for qc in range(2):
                            nc.tensor.matmul(
                                ps[:, qc * 512:(qc + 1) * 512],
                                lhsT=ktz[po:po + 64, slot, kc * P:(kc + 1) * P],
                                rhs=qtz[po:po + 64, slot, qc * 512:(qc + 1) * 512])
                        es = esp.tile([P, NQ], BF16, tag="es")
                        nc.scalar.activation(es[:], ps[:], EXP, scale=SCALE)
                        es_hist[kc] = es
                    emit_pv(NKC - 2)
                    emit_pv(NKC - 1)
                    nc.vector.tensor_tensor(pvacc[:], pv[:], pvacc[:], ADD)

                    # normalization
                    rec = rrp.tile([1, NQ], F32, tag="rec")
                    nc.vector.reciprocal(rec[:], pvacc[HD:HD + 1, :])
                    rb = rrp.tile([64, NQ], F32, tag="rb")
                    nc.gpsimd.partition_broadcast(rb[:], rec[:])
                    nc.vector.tensor_tensor(
                        attnT[po:po + 64, hp, :],
                        pvacc[0:HD, :], rb[:], MULT)
                while fillers:
                    fillers.pop(0)()

            # ---- tail: out = attnT^T @ w_o + b_o ----
            for q8 in range(NQ // P):
                if q8 % 3 == 2:
                    ps = ppvp.tile([P, NQ], F32, tag="pv")
                else:
                    ps = pscp.tile([P, NQ], F32, tag="sc")
                for oc2 in range(2):
                    for dc in range(DC):
                        nc.tensor.matmul(
                            ps[:, oc2 * 512:(oc2 + 1) * 512],
                            lhsT=attnT[:, dc, q8 * P:(q8 + 1) * P],
                            rhs=wo_sb[:, dc, oc2 * 512:(oc2 + 1) * 512],
                            start=(dc == 0), stop=(dc == DC - 1))
                for oc2 in range(2):
                    ot = otp.tile([P, 512], F32, tag="ot")
                    nc.vector.tensor_tensor(
                        ot[:], ps[:, oc2 * 512:(oc2 + 1) * 512],
                        bo_bc[:, oc2 * 512:(oc2 + 1) * 512], ADD)
                    nc.sync.dma_start(
                        out_d[q8 * P:(q8 + 1) * P,
                              oc2 * 512:(oc2 + 1) * 512], ot[:])

    nc.compile()
    return nc


_NC_CACHE = None


def _get_nc():
    global _NC_CACHE
    if _NC_CACHE is None:
        _NC_CACHE = _build()
    return _NC_CACHE


def _run(z, w_q, w_k, w_v, w_o, b_o, **spmd_kwargs):
    z = np.ascontiguousarray(np.asarray(z, dtype=np.float32))
    w_q = np.ascontiguousarray(np.asarray(w_q, dtype=np.float32)).astype(
        ml_dtypes.bfloat16)
    w_k = np.ascontiguousarray(np.asarray(w_k, dtype=np.float32)).astype(
        ml_dtypes.bfloat16)
    w_v = np.ascontiguousarray(np.asarray(w_v, dtype=np.float32)).astype(
        ml_dtypes.bfloat16)
    w_o = np.ascontiguousarray(np.asarray(w_o, dtype=np.float32)).astype(
        ml_dtypes.bfloat16)
    b_o = np.ascontiguousarray(np.asarray(b_o, dtype=np.float32))
    assert z.shape == (B, N, D)

    if not spmd_kwargs.get("trace"):
        # A stray BASS_TRACE in the environment would route through the NTFF
        # hook (absent in this image) and crash; force the no-trace path.
        os.environ["BASS_NEVER_TRACE"] = "1"

    nc = _get_nc()
    in_maps = []
    for c in range(N_CORES):
        b = c // 2
        off = (c % 2) * NQ
        zc = np.ascontiguousarray(np.concatenate([z[b, off:], z[b, :off]], axis=0))
        in_maps.append({"z": zc, "w_q": w_q, "w_k": w_k, "w_v": w_v,
                        "w_o": w_o, "b_o": b_o})

    res = run_bass_kernel_spmd(nc, in_maps, core_ids=list(range(N_CORES)),
                               **spmd_kwargs)
    out = np.empty((B, N, D), dtype=np.float32)
    for c in range(N_CORES):
        b = c // 2
        off = (c % 2) * NQ
        out[b, off:off + NQ, :] = res.results[c]["out"]
    return out, res


def kernel(z, w_q, w_k, w_v, w_o, b_o):
    out, _ = _run(z, w_q, w_k, w_v, w_o, b_o)
    return out
